# revision 11
# baseline (speedup 1.0000x reference)
"""Trainium2 Bass kernel for nn_AdvectionDiffusionReaction2M (v4).

Advection-diffusion-reaction on a 512x512 grid, 199 sequential steps, output =
all intermediate states (199,512,512) f32.

Sharding: rows split 8 ways (64 rows/core) with 16-row ghost zones refreshed
by an AllGather every 16 steps.  SBUF layout per core: flat [128, 6B] f32
per state buffer:
    [ GL (B) | b1 b2 b3 b4 (4B) | GR (B) ]
partition p = column group (cols 4p..4p+3 at blocks b1..b4), GL/GR = ghost
columns 4p-1 / 4p+4, i = stored row (96 = 16 ghost + 64 + 16 ghost).

v4 vs v3: the pads between GL/blocks/GR are removed so the W-neighbor stream
for CLIN is the single contiguous window [GL|b1|b2|b3] and the E-stream for
DLIN is [b2|b3|b4|GR] -- one custom op per neighbor instead of interior+ghost
pairs.  The Up/Dn windows ([B+lo-1, 4B+hi-1] etc.) never touch GL/GR because
the valid-row window always has lo >= 1, so the ghost-column refresh (PE
partition-shift matmuls + ACT PSUM->SBUF copies) still overlaps the leading
DVE ops of the next step.  All boundary fixups run on the DVE in a single
uniform order (SELif -> SEL511 -> col0 -> rows) that is correct for every
step including t=1.

The update is regrouped per neighbor with Tc-dependent coefficients
   Tn = Up*(s+h*Tc^2) + Dn*(s-h*Tc^2) + L*(s-h*Tc) + R*(s+h*Tc) + phi(Tc)
   phi = Tc + g*(Tc^3-Tc^2+Tc),  g = h*2dx
computed by fused custom DVE ops (block-edge rows are sacrificial ghost rows,
so row-crossing garbage in Up/Dn is harmless).
"""

import os
import numpy as np

N = 512
DX = 1.0 / (N - 1)
DT = 1e-7
MB = 256
NCORES = 8
K = 16                      # ghost depth (rows)
RS = 64 + 2 * K             # stored rows per core (96)
NSTEPS = int(os.environ.get("ADR_NSTEPS", "199"))
B = RS                      # block stride in flat free dim
F = 6 * B                   # flat state width: GL|b1..b4|GR

LAST_EXEC_NS = None
LAST_RESULT = None

_OPS_REGISTERED = {}


def _register_ops():
    """Register custom DVE ops (runtime registration into dve_ops.OPS)."""
    if _OPS_REGISTERED:
        return _OPS_REGISTERED
    import concourse.dve_ops as dve_ops
    from concourse.dve_ops import DveOp, OPS
    from concourse.dve_spec import Spec, Src0, Src1, C0, C1, C2, One, sq, lower
    from concourse.dve_uop import DveOpSpec

    def make_op(name, body, reference):
        for op in OPS:
            if op.name == name:
                return op
        spec = Spec(body=body, reference=reference)
        shas = {}
        for ver in ("v3", "v4"):
            uops = lower(spec, ver=ver)
            tmp = DveOpSpec(name=name, opcode=0, uops=uops, rd1_en=True)
            shas[ver] = tmp.sha(ver)
        op = DveOp(name, spec, subdim=False, uops_sha=shas)
        OPS.append(op)
        dve_ops._SUB_OPCODE_FOR_NAME[name] = (
            dve_ops._CUSTOM_DVE_ROW_BASE + len(OPS) - 1)
        assert dve_ops._SUB_OPCODE_FOR_NAME[name] < 0x20, "opcode row overflow"
        dve_ops.CUSTOM_DVE_SPECS[name] = spec
        return op

    q = sq(Src0)
    gc = C0 * C2                          # g = h * 2dx (hoisted mult)
    # out = Up*(s + h*Tc^2) + g*(Tc^2 - Tc)*Tc      [phi part 1: g(Tc^3-Tc^2)]
    _OPS_REGISTERED["APHI"] = make_op(
        "ADR_APHI",
        Src1 * (C1 + q * C0) + (q - Src0) * gc * Src0,
        lambda in0, in1, s0, s1, imm2:
            in1 * (s1 + in0**2 * s0)
            + (in0**2 - in0) * (s0 * imm2) * in0)
    # out = Dn*(s - h*Tc^2)
    _OPS_REGISTERED["BSQ"] = make_op(
        "ADR_BSQ", Src1 * (C1 - q * C0),
        lambda in0, in1, s0, s1: in1 * (s1 - in0**2 * s0))
    # out = L*(s - h*Tc) + (h*Tc)*2dx               [phi part 3: g*Tc]
    _a = Src0 * C0
    _OPS_REGISTERED["CLIN"] = make_op(
        "ADR_CLIN", Src1 * (C1 - _a) + _a * C2,
        lambda in0, in1, s0, s1, imm2:
            in1 * (s1 - in0 * s0) + in0 * s0 * imm2)
    # out = R*(s + h*Tc) + Tc + (-4)*s*Tc           [phi part 2: (1-4s)Tc]
    _OPS_REGISTERED["DLIN"] = make_op(
        "ADR_DLIN", Src1 * (C1 + _a) + Src0 + Src0 * C1 * C2,
        lambda in0, in1, s0, s1, imm2:
            in1 * (s1 + in0 * s0) + in0 + in0 * s1 * imm2)
    # out = Src0*C0 + Src1*C1  (masked blend / select)
    _OPS_REGISTERED["SEL"] = make_op(
        "ADR_SEL", Src0 * C0 + Src1 * C1,
        lambda in0, in1, s0, s1: in0 * s0 + in1 * s1)
    return _OPS_REGISTERED


def _pack_core(G, c):
    """Full grid (512,512) -> per-core flat tile [128, F] (f32, zero padded).

    Layout per partition p: [GL | b1 b2 b3 b4 | GR] where block bj holds
    column 4p+j-1 over the RS stored rows and GL/GR hold cols 4p-1 / 4p+4.
    """
    lo = 64 * c - K
    S = np.zeros((RS, N), np.float32)
    g0, g1 = max(lo, 0), min(lo + RS, N)
    S[g0 - lo: g1 - lo] = G[g0:g1]
    cols = (4 * np.arange(128)[:, None] - 1 + np.arange(6)[None, :])  # [128,6]
    valid = (cols >= 0) & (cols < N)
    t = S.T[np.clip(cols, 0, N - 1)]          # [128, 6, RS]
    t[~valid] = 0.0
    flat = np.zeros((128, F), np.float32)
    flat[:, 0:B] = t[:, 0]                      # GL
    flat[:, B:5 * B] = t[:, 1:5].reshape(128, 4 * B)
    flat[:, 5 * B:6 * B] = t[:, 5]              # GR
    return np.ascontiguousarray(flat, dtype=np.float32)


def _build(nc, tile, mybir, bass):
    f32 = mybir.dt.float32
    u32 = mybir.dt.uint32
    bf16 = mybir.dt.bfloat16
    OP = mybir.AluOpType
    ops = _register_ops()
    APHI, BSQ, CLIN, DLIN, SEL = (ops[k] for k in
                                  ("APHI", "BSQ", "CLIN", "DLIN", "SEL"))

    u0s_d = nc.dram_tensor("u0s", [128, F], f32, kind="ExternalInput").ap()
    ppc_d = nc.dram_tensor("ppc", [128, 16], f32, kind="ExternalInput").ap()
    wr_d = nc.dram_tensor("wr", [128, 128], f32, kind="ExternalInput").ap()
    wl_d = nc.dram_tensor("wl", [128, 128], f32, kind="ExternalInput").ap()
    nbrs_d = nc.dram_tensor("nbrs", [1, 2], u32, kind="ExternalInput").ap()
    rsel_d = nc.dram_tensor("rsel", [1, 2], u32, kind="ExternalInput").ap()
    u8 = mybir.dt.uint8
    msk_d = nc.dram_tensor("msk", [128, B], u8, kind="ExternalInput").ap()
    out_d = nc.dram_tensor("out", [NSTEPS, 128, 4, 64], f32,
                           kind="ExternalOutput").ap()

    # ghost sync every K steps (synchronous: state-t bands must merge into the
    # state-t tile before step t+1 -- any lag breaks time-consistency)
    nsync = [t for t in range(K, NSTEPS, K)]

    with tile.TileContext(nc) as tc:
        with tc.tile_pool(name="state", bufs=1) as sp, \
             tc.tile_pool(name="tmp", bufs=3) as tp, \
             tc.tile_pool(name="psum", bufs=4, space="PSUM") as pp, \
             tc.tile_pool(name="dram", bufs=1, space="DRAM") as dp:

            tA = sp.tile([128, F], f32, tag="tA")
            tB = sp.tile([128, F], f32, tag="tB")
            tC = sp.tile([128, F], f32, tag="tC")
            ppc = sp.tile([128, 16], f32, tag="ppc")
            wr = sp.tile([128, 128], f32, tag="wr")
            wl = sp.tile([128, 128], f32, tag="wl")

            cc_in = dp.tile([2, 128, 4, K], f32, tag="ccin")
            cc_outs = {t: dp.tile([16 * 128, 4, K], f32, tag=f"ccout{t}",
                                  name=f"ccout{t}", addr_space="Shared")
                       for t in nsync}

            nc.sync.dma_start(tA[:], u0s_d[:])
            nc.sync.dma_start(ppc[:], ppc_d[:])
            nc.sync.dma_start(wr[:], wr_d[:])
            nc.sync.dma_start(wl[:], wl_d[:])

            rp = nc.alloc_registers("rprev")
            nc.regs_load(rp, nbrs_d[0:1, 0:1])
            sv_prev = nc.snap(rp, min_val=0, max_val=15 * 128)
            rn = nc.alloc_registers("rnext")
            nc.regs_load(rn, nbrs_d[0:1, 1:2])
            sv_next = nc.snap(rn, min_val=0, max_val=15 * 128)
            # per-core Neumann source rows (core 0: 17 else 16; core 7: 78 else 79)
            rt0 = nc.alloc_registers("rtop")
            nc.regs_load(rt0, rsel_d[0:1, 0:1])
            sv_rtop = nc.snap(rt0, min_val=K, max_val=K + 1)
            rb0 = nc.alloc_registers("rbot")
            nc.regs_load(rb0, rsel_d[0:1, 1:2])
            sv_rbot = nc.snap(rb0, min_val=K + 62, max_val=K + 63)

            s_ = ppc[:, 3:4]; h_ = ppc[:, 4:5]
            ifB = ppc[:, 10:11]; ifC = ppc[:, 11:12]
            cD = ppc[:, 12:13]
            rab = ppc[:, 15:16]

            # warm-up AllGather (tiny payload): pays the CC cold-start cost
            # while the first steps compute, so the step-16 collective runs
            # closer to warm latency.
            warm_in = dp.tile([1, 64], f32, tag="warmin")
            warm_out = dp.tile([8, 64], f32, tag="warmout", name="warmout",
                               addr_space="Shared")
            nc.gpsimd.collective_compute(
                "AllGather", OP.bypass,
                replica_groups=[list(range(NCORES))],
                ins=[warm_in[:]], outs=[warm_out[:]])

            # mask tile for copy_predicated col-511 fix: m127 broadcast
            mskt = sp.tile([128, B], mybir.dt.uint8, tag="mskt")
            nc.sync.dma_start(mskt[:], msk_d[:])

            # triple-buffered state: the output DMA of step t reads buffer
            # written at t; with only two buffers step t+2's TnV hits a WAR
            # wait on that DMA.  Three buffers give it two extra steps of
            # slack.
            bufs3 = [tA, tB, tC]
            for t in range(1, NSTEPS + 1):
                cur = bufs3[(t - 1) % 3]
                nxt = bufs3[t % 3]
                # ghost rows decay one row per step since the last refresh:
                # only rows [lo, hi) need computing this step.  Flat windows
                # span all 4 blocks; rows outside [lo, hi) in interior blocks
                # get garbage, which is harmless (they are decayed ghost rows
                # never read again before the next merge overwrites them).
                m = ((t - 1) % K) + 1
                lo, hi = m, RS - m
                w = 3 * B + hi - lo          # width of the 4-block out window

                QU = tp.tile([128, 4 * B], bf16, tag="QU")
                QD = tp.tile([128, 4 * B], bf16, tag="QD")
                QL = tp.tile([128, 4 * B], bf16, tag="QL")
                PR = tp.tile([128, 4 * B], f32, tag="PR")
                S1 = tp.tile([128, 4 * B], bf16, tag="S1")
                S2 = tp.tile([128, 4 * B], bf16, tag="S2")
                I1 = tp.tile([128, B], f32, tag="I1")

                # neighbor-grouped fused passes (DVE), flat 4-block windows
                nc.vector._custom_dve(APHI,
                                      out=QU[:, lo:w + lo],
                                      in0=cur[:, B + lo:B + lo + w],
                                      in1=cur[:, B + lo - 1:B + lo - 1 + w],
                                      s0=h_, s1=s_, imm2=2.0 * DX)
                nc.vector._custom_dve(BSQ,
                                      out=QD[:, lo:w + lo],
                                      in0=cur[:, B + lo:B + lo + w],
                                      in1=cur[:, B + lo + 1:B + lo + 1 + w],
                                      s0=h_, s1=s_)
                # interface precompute (reads OLD state only):
                # X = (ca/cb)*GR + b3; SELif scales by ifB = cb*m63 so
                # b4[63] gets ca*GR + cb*b3.
                nc.vector.scalar_tensor_tensor(
                    I1[:, lo:hi],
                    cur[:, 5 * B + lo:5 * B + hi], rab,
                    cur[:, 3 * B + lo:3 * B + hi],
                    OP.mult, OP.add)

                # S1 placed before CLIN/DLIN so the previous step's ghost
                # column refresh (PE+ACT) has extra slack before CLIN reads GL
                nc.vector.tensor_tensor(S1[:, lo:w + lo], QU[:, lo:w + lo],
                                        QD[:, lo:w + lo], OP.add)
                # CLIN: W stream = [GL|b1|b2|b3] contiguous
                nc.vector._custom_dve(CLIN,
                                      out=QL[:, lo:w + lo],
                                      in0=cur[:, B + lo:B + lo + w],
                                      in1=cur[:, lo:lo + w],
                                      s0=h_, s1=s_, imm2=2.0 * DX)
                # DLIN: E stream = [b2|b3|b4|GR] contiguous; carries phi
                # (includes raw Tc -> f32 output)
                nc.vector._custom_dve(DLIN,
                                      out=PR[:, lo:w + lo],
                                      in0=cur[:, B + lo:B + lo + w],
                                      in1=cur[:, 2 * B + lo:2 * B + lo + w],
                                      s0=h_, s1=s_, imm2=-4.0)
                nc.vector.tensor_tensor(S2[:, lo:w + lo], S1[:, lo:w + lo],
                                        QL[:, lo:w + lo], OP.add)
                nc.vector.tensor_tensor(nxt[:, B + lo:B + lo + w],
                                        S2[:, lo:w + lo], PR[:, lo:w + lo],
                                        OP.add)

                # boundary fixups, uniform order for every step: interface
                # blend and col-511 masked copy on DVE, col-0 copy on ACT
                # (hidden behind the DVE fixup tail), row copies on DVE.
                b4v = nxt[:, 4 * B + lo:4 * B + hi]
                nc.scalar.copy(nxt[0:1, B + lo:B + hi],
                               nxt[0:1, 2 * B + lo:2 * B + hi])
                nc.vector._custom_dve(SEL, out=b4v, in0=b4v,
                                      in1=I1[:, lo:hi], s0=ifC, s1=ifB)
                nc.vector.copy_predicated(b4v, mskt[:, lo:hi],
                                          nxt[:, 3 * B + lo:3 * B + hi])
                nx4 = nxt[:, B:5 * B].rearrange("p (b i) -> p b i", b=4)
                nc.vector.tensor_scalar(nx4[:, :, K:K + 1],
                                        nx4[:, :, bass.ds(sv_rtop, 1)],
                                        1.0, None, OP.mult)
                nc.vector.tensor_scalar(nx4[:, :, K + 63:K + 64],
                                        nx4[:, :, bass.ds(sv_rbot, 1)],
                                        1.0, None, OP.mult)

                # ghost row sync (blocking; gpsimd queue keeps Sync free).
                # Bands carry only the 4 state blocks; GL/GR are rebuilt by
                # the ghost-column matmuls placed AFTER the merge below.
                if t in nsync:
                    cc_out = cc_outs[t]
                    nc.sync.dma_start(cc_in[0], nx4[:, :, K:2 * K])
                    nc.sync.dma_start(cc_in[1], nx4[:, :, 64:64 + K])
                    nc.gpsimd.collective_compute(
                        "AllGather", OP.bypass,
                        replica_groups=[list(range(NCORES))],
                        ins=[cc_in[:]], outs=[cc_out[:]])
                    nc.gpsimd.dma_start(nx4[:, :, 0:K],
                                        cc_out[bass.ds(sv_prev, 128)])
                    nc.sync.dma_start(nx4[:, :, 64 + K:64 + 2 * K],
                                      cc_out[bass.ds(sv_next, 128)])

                # ghost column refresh via partition-shift matmuls, windowed
                # to the rows step t+1 will read.  At sync steps this reads
                # the merged tile, so the new GL/GR include fresh ghost rows.
                if t < NSTEPS:
                    m2 = (t % K) + 1
                    l2, h2 = m2, RS - m2
                    psR = pp.tile([128, B], f32, tag="psR")
                    psL = pp.tile([128, B], f32, tag="psL")
                    nc.tensor.matmul(psL[:, l2:h2], wl[:],
                                     nxt[:, 4 * B + l2:4 * B + h2],
                                     start=True, stop=True)
                    nc.tensor.matmul(psR[:, l2:h2], wr[:],
                                     nxt[:, B + l2:B + h2],
                                     start=True, stop=True)
                    nc.scalar.copy(nxt[:, l2:h2], psL[:, l2:h2])
                    nc.scalar.copy(nxt[:, 5 * B + l2:5 * B + h2],
                                   psR[:, l2:h2])

                # output: owned rows (the read has three steps of slack)
                nc.sync.dma_start(out_d[t - 1], nx4[:, :, K:K + 64])
    return nc


def _ensure_ntff_hook():
    """Provide antenv.axon_hooks (missing in this image) so bass_utils can
    NTFF-profile under axon."""
    import sys
    import types
    try:
        from antenv.axon_hooks import get_axon_ntff_profile_hook  # noqa: F401
        return
    except ImportError:
        pass
    mod = types.ModuleType("antenv.axon_hooks")
    mod._hook = None

    def set_axon_ntff_profile_hook(h):
        mod._hook = h

    def get_axon_ntff_profile_hook():
        return mod._hook

    mod.set_axon_ntff_profile_hook = set_axon_ntff_profile_hook
    mod.get_axon_ntff_profile_hook = get_axon_ntff_profile_hook
    sys.modules["antenv.axon_hooks"] = mod
    import antenv
    antenv.axon_hooks = mod
    try:
        from trn_agent_boot.trn_boot import _ntff_profile_via_ctypes
        hook = _ntff_profile_via_ctypes("/opt/axon/libaxon_pjrt.so")
        if hook is not None:
            mod._hook = hook
    except Exception:
        pass


def kernel(u0, k1, k2, alpha1, alpha2):
    global LAST_EXEC_NS, LAST_RESULT
    import concourse.bacc as bacc
    import concourse.bass as bass
    import concourse.tile as tile
    import concourse.mybir as mybir
    from concourse.bass_utils import run_bass_kernel_spmd

    u0 = np.asarray(u0, dtype=np.float32)
    k1f = float(np.asarray(k1).reshape(-1)[0])
    k2f = float(np.asarray(k2).reshape(-1)[0])
    a1f = float(np.asarray(alpha1).reshape(-1)[0])
    a2f = float(np.asarray(alpha2).reshape(-1)[0])

    dx2 = DX * DX
    ca, cb = k1f / (k1f + k2f), k2f / (k1f + k2f)

    nc = bacc.Bacc(
        "TRN2", target_bir_lowering=False, debug=False,
        num_devices=NCORES,
    )
    _build(nc, tile, mybir, bass)
    nc.compile()

    left = np.arange(128) < 64
    s = np.where(left, DT * a1f / dx2, DT * a2f / dx2).astype(np.float32)
    h = np.where(left, DT * k1f / (2 * DX), DT * k2f / (2 * DX)).astype(np.float32)
    WR = np.eye(128, k=-1, dtype=np.float32)   # out[m] = in[m+1]
    WL = np.eye(128, k=+1, dtype=np.float32)   # out[m] = in[m-1]

    m63 = (np.arange(128) == 63).astype(np.float32)
    m127 = (np.arange(128) == 127).astype(np.float32)
    in_maps = []
    for c in range(NCORES):
        ppc = np.zeros((128, 16), np.float32)
        ppc[:, 3] = s
        ppc[:, 4] = h
        ppc[:, 10] = m63 * cb              # ifB
        ppc[:, 11] = 1.0 - m63             # ifC
        ppc[:, 12] = m127                  # cD (copy_predicated mask source)
        ppc[:, 15] = ca / cb               # rab (interface ratio)
        prev_off = (2 * (c - 1) + 1) * 128 if c > 0 else 0
        next_off = (2 * (c + 1)) * 128 if c < NCORES - 1 else 0
        rtop = K + 1 if c == 0 else K
        rbot = K + 62 if c == NCORES - 1 else K + 63
        in_maps.append({
            "u0s": _pack_core(u0, c),
            "ppc": ppc,
            "wr": WR,
            "wl": WL,
            "nbrs": np.array([[prev_off, next_off]], dtype=np.uint32),
            "rsel": np.array([[rtop, rbot]], dtype=np.uint32),
            "msk": np.broadcast_to(m127[:, None].astype(np.uint8),
                                   (128, B)).copy(),
        })

    trace = os.environ.get("ADR_TRACE", "0") == "1"
    if trace:
        _ensure_ntff_hook()
    res = run_bass_kernel_spmd(
        nc, in_maps, core_ids=list(range(NCORES)), trace=trace)
    LAST_EXEC_NS = res.exec_time_ns
    LAST_RESULT = res

    full = np.zeros((NSTEPS, N, N), np.float32)
    for c in range(NCORES):
        arr = np.asarray(res.results[c]["out"]).reshape(NSTEPS, 128, 4, 64)
        full[:, 64 * c:64 * (c + 1), :] = (
            arr.transpose(0, 3, 1, 2).reshape(NSTEPS, 64, 512))
    return full


# revision 12
# speedup vs baseline: 1.0015x; 1.0015x over previous
"""Trainium2 Bass kernel for nn_AdvectionDiffusionReaction2M (v4).

Advection-diffusion-reaction on a 512x512 grid, 199 sequential steps, output =
all intermediate states (199,512,512) f32.

Sharding: rows split 8 ways (64 rows/core) with 16-row ghost zones refreshed
by an AllGather every 16 steps.  SBUF layout per core: flat [128, 6B] f32
per state buffer:
    [ GL (B) | b1 b2 b3 b4 (4B) | GR (B) ]
partition p = column group (cols 4p..4p+3 at blocks b1..b4), GL/GR = ghost
columns 4p-1 / 4p+4, i = stored row (96 = 16 ghost + 64 + 16 ghost).

v4 vs v3: the pads between GL/blocks/GR are removed so the W-neighbor stream
for CLIN is the single contiguous window [GL|b1|b2|b3] and the E-stream for
DLIN is [b2|b3|b4|GR] -- one custom op per neighbor instead of interior+ghost
pairs.  The Up/Dn windows ([B+lo-1, 4B+hi-1] etc.) never touch GL/GR because
the valid-row window always has lo >= 1, so the ghost-column refresh (PE
partition-shift matmuls + ACT PSUM->SBUF copies) still overlaps the leading
DVE ops of the next step.  All boundary fixups run on the DVE in a single
uniform order (SELif -> SEL511 -> col0 -> rows) that is correct for every
step including t=1.

The update is regrouped per neighbor with Tc-dependent coefficients
   Tn = Up*(s+h*Tc^2) + Dn*(s-h*Tc^2) + L*(s-h*Tc) + R*(s+h*Tc) + phi(Tc)
   phi = Tc + g*(Tc^3-Tc^2+Tc),  g = h*2dx
computed by fused custom DVE ops (block-edge rows are sacrificial ghost rows,
so row-crossing garbage in Up/Dn is harmless).
"""

import os
import numpy as np

N = 512
DX = 1.0 / (N - 1)
DT = 1e-7
MB = 256
NCORES = 8
K = 16                      # ghost depth (rows)
RS = 64 + 2 * K             # stored rows per core (96)
NSTEPS = int(os.environ.get("ADR_NSTEPS", "199"))
B = RS                      # block stride in flat free dim
F = 6 * B                   # flat state width: GL|b1..b4|GR

LAST_EXEC_NS = None
LAST_RESULT = None

_OPS_REGISTERED = {}


def _register_ops():
    """Register custom DVE ops (runtime registration into dve_ops.OPS)."""
    if _OPS_REGISTERED:
        return _OPS_REGISTERED
    import concourse.dve_ops as dve_ops
    from concourse.dve_ops import DveOp, OPS
    from concourse.dve_spec import Spec, Src0, Src1, C0, C1, C2, One, sq, lower
    from concourse.dve_uop import DveOpSpec

    def make_op(name, body, reference):
        for op in OPS:
            if op.name == name:
                return op
        spec = Spec(body=body, reference=reference)
        shas = {}
        for ver in ("v3", "v4"):
            uops = lower(spec, ver=ver)
            tmp = DveOpSpec(name=name, opcode=0, uops=uops, rd1_en=True)
            shas[ver] = tmp.sha(ver)
        op = DveOp(name, spec, subdim=False, uops_sha=shas)
        OPS.append(op)
        dve_ops._SUB_OPCODE_FOR_NAME[name] = (
            dve_ops._CUSTOM_DVE_ROW_BASE + len(OPS) - 1)
        assert dve_ops._SUB_OPCODE_FOR_NAME[name] < 0x20, "opcode row overflow"
        dve_ops.CUSTOM_DVE_SPECS[name] = spec
        return op

    q = sq(Src0)
    gc = C0 * C2                          # g = h * 2dx (hoisted mult)
    # out = Up*(s + h*Tc^2) + g*(Tc^2 - Tc)*Tc      [phi part 1: g(Tc^3-Tc^2)]
    _OPS_REGISTERED["APHI"] = make_op(
        "ADR_APHI",
        Src1 * (C1 + q * C0) + (q - Src0) * gc * Src0,
        lambda in0, in1, s0, s1, imm2:
            in1 * (s1 + in0**2 * s0)
            + (in0**2 - in0) * (s0 * imm2) * in0)
    # out = Dn*(s - h*Tc^2)
    _OPS_REGISTERED["BSQ"] = make_op(
        "ADR_BSQ", Src1 * (C1 - q * C0),
        lambda in0, in1, s0, s1: in1 * (s1 - in0**2 * s0))
    # out = L*(s - h*Tc) + (h*Tc)*2dx               [phi part 3: g*Tc]
    _a = Src0 * C0
    _OPS_REGISTERED["CLIN"] = make_op(
        "ADR_CLIN", Src1 * (C1 - _a) + _a * C2,
        lambda in0, in1, s0, s1, imm2:
            in1 * (s1 - in0 * s0) + in0 * s0 * imm2)
    # out = R*(s + h*Tc) + Tc + (-4)*s*Tc           [phi part 2: (1-4s)Tc]
    _OPS_REGISTERED["DLIN"] = make_op(
        "ADR_DLIN", Src1 * (C1 + _a) + Src0 + Src0 * C1 * C2,
        lambda in0, in1, s0, s1, imm2:
            in1 * (s1 + in0 * s0) + in0 + in0 * s1 * imm2)
    # out = Src0*C0 + Src1*C1  (masked blend / select)
    _OPS_REGISTERED["SEL"] = make_op(
        "ADR_SEL", Src0 * C0 + Src1 * C1,
        lambda in0, in1, s0, s1: in0 * s0 + in1 * s1)
    return _OPS_REGISTERED


def _pack_core(G, c):
    """Full grid (512,512) -> per-core flat tile [128, F] (f32, zero padded).

    Layout per partition p: [GL | b1 b2 b3 b4 | GR] where block bj holds
    column 4p+j-1 over the RS stored rows and GL/GR hold cols 4p-1 / 4p+4.
    """
    lo = 64 * c - K
    S = np.zeros((RS, N), np.float32)
    g0, g1 = max(lo, 0), min(lo + RS, N)
    S[g0 - lo: g1 - lo] = G[g0:g1]
    cols = (4 * np.arange(128)[:, None] - 1 + np.arange(6)[None, :])  # [128,6]
    valid = (cols >= 0) & (cols < N)
    t = S.T[np.clip(cols, 0, N - 1)]          # [128, 6, RS]
    t[~valid] = 0.0
    flat = np.zeros((128, F), np.float32)
    flat[:, 0:B] = t[:, 0]                      # GL
    flat[:, B:5 * B] = t[:, 1:5].reshape(128, 4 * B)
    flat[:, 5 * B:6 * B] = t[:, 5]              # GR
    return np.ascontiguousarray(flat, dtype=np.float32)


def _build(nc, tile, mybir, bass):
    f32 = mybir.dt.float32
    u32 = mybir.dt.uint32
    bf16 = mybir.dt.bfloat16
    OP = mybir.AluOpType
    ops = _register_ops()
    APHI, BSQ, CLIN, DLIN, SEL = (ops[k] for k in
                                  ("APHI", "BSQ", "CLIN", "DLIN", "SEL"))

    u0s_d = nc.dram_tensor("u0s", [128, F], f32, kind="ExternalInput").ap()
    ppc_d = nc.dram_tensor("ppc", [128, 16], f32, kind="ExternalInput").ap()
    wr_d = nc.dram_tensor("wr", [128, 128], f32, kind="ExternalInput").ap()
    wl_d = nc.dram_tensor("wl", [128, 128], f32, kind="ExternalInput").ap()
    nbrs_d = nc.dram_tensor("nbrs", [1, 2], u32, kind="ExternalInput").ap()
    rsel_d = nc.dram_tensor("rsel", [1, 2], u32, kind="ExternalInput").ap()
    u8 = mybir.dt.uint8
    msk_d = nc.dram_tensor("msk", [128, B], u8, kind="ExternalInput").ap()
    out_d = nc.dram_tensor("out", [NSTEPS, 128, 4, 64], f32,
                           kind="ExternalOutput").ap()

    # ghost sync every K steps (synchronous: state-t bands must merge into the
    # state-t tile before step t+1 -- any lag breaks time-consistency)
    nsync = [t for t in range(K, NSTEPS, K)]

    with tile.TileContext(nc) as tc:
        with tc.tile_pool(name="state", bufs=1) as sp, \
             tc.tile_pool(name="tmp", bufs=2) as tp, \
             tc.tile_pool(name="psum", bufs=2, space="PSUM") as pp, \
             tc.tile_pool(name="dram", bufs=1, space="DRAM") as dp:

            tA = sp.tile([128, F], f32, tag="tA")
            tB = sp.tile([128, F], f32, tag="tB")
            tC = sp.tile([128, F], f32, tag="tC")
            ppc = sp.tile([128, 16], f32, tag="ppc")
            wr = sp.tile([128, 128], f32, tag="wr")
            wl = sp.tile([128, 128], f32, tag="wl")

            cc_in = dp.tile([2, 128, 4, K], f32, tag="ccin")
            cc_outs = {t: dp.tile([16 * 128, 4, K], f32, tag=f"ccout{t}",
                                  name=f"ccout{t}", addr_space="Shared")
                       for t in nsync}

            nc.sync.dma_start(tA[:], u0s_d[:])
            nc.sync.dma_start(ppc[:], ppc_d[:])
            nc.sync.dma_start(wr[:], wr_d[:])
            nc.sync.dma_start(wl[:], wl_d[:])

            rp = nc.alloc_registers("rprev")
            nc.regs_load(rp, nbrs_d[0:1, 0:1])
            sv_prev = nc.snap(rp, min_val=0, max_val=15 * 128)
            rn = nc.alloc_registers("rnext")
            nc.regs_load(rn, nbrs_d[0:1, 1:2])
            sv_next = nc.snap(rn, min_val=0, max_val=15 * 128)
            # per-core Neumann source rows (core 0: 17 else 16; core 7: 78 else 79)
            rt0 = nc.alloc_registers("rtop")
            nc.regs_load(rt0, rsel_d[0:1, 0:1])
            sv_rtop = nc.snap(rt0, min_val=K, max_val=K + 1)
            rb0 = nc.alloc_registers("rbot")
            nc.regs_load(rb0, rsel_d[0:1, 1:2])
            sv_rbot = nc.snap(rb0, min_val=K + 62, max_val=K + 63)

            s_ = ppc[:, 3:4]; h_ = ppc[:, 4:5]
            ifB = ppc[:, 10:11]; ifC = ppc[:, 11:12]
            cD = ppc[:, 12:13]
            rab = ppc[:, 15:16]

            # warm-up AllGather (tiny payload): pays the CC cold-start cost
            # while the first steps compute, so the step-16 collective runs
            # closer to warm latency.
            warm_in = dp.tile([1, 64], f32, tag="warmin")
            warm_out = dp.tile([8, 64], f32, tag="warmout", name="warmout",
                               addr_space="Shared")
            nc.gpsimd.collective_compute(
                "AllGather", OP.bypass,
                replica_groups=[list(range(NCORES))],
                ins=[warm_in[:]], outs=[warm_out[:]])

            # mask tile for copy_predicated col-511 fix: m127 broadcast
            mskt = sp.tile([128, B], mybir.dt.uint8, tag="mskt")
            nc.sync.dma_start(mskt[:], msk_d[:])

            # triple-buffered state: the output DMA of step t reads buffer
            # written at t; with only two buffers step t+2's TnV hits a WAR
            # wait on that DMA.  Three buffers give it two extra steps of
            # slack.
            bufs3 = [tA, tB, tC]
            for t in range(1, NSTEPS + 1):
                cur = bufs3[(t - 1) % 3]
                nxt = bufs3[t % 3]
                # ghost rows decay one row per step since the last refresh:
                # only rows [lo, hi) need computing this step.  Flat windows
                # span all 4 blocks; rows outside [lo, hi) in interior blocks
                # get garbage, which is harmless (they are decayed ghost rows
                # never read again before the next merge overwrites them).
                m = ((t - 1) % K) + 1
                lo, hi = m, RS - m
                w = 3 * B + hi - lo          # width of the 4-block out window

                QU = tp.tile([128, 4 * B], bf16, tag="QU")
                QD = tp.tile([128, 4 * B], bf16, tag="QD")
                QL = tp.tile([128, 4 * B], bf16, tag="QL")
                PR = tp.tile([128, 4 * B], f32, tag="PR")
                S1 = tp.tile([128, 4 * B], bf16, tag="S1")
                S2 = tp.tile([128, 4 * B], bf16, tag="S2")
                I1 = tp.tile([128, B], f32, tag="I1")

                # neighbor-grouped fused passes (DVE), flat 4-block windows
                nc.vector._custom_dve(APHI,
                                      out=QU[:, lo:w + lo],
                                      in0=cur[:, B + lo:B + lo + w],
                                      in1=cur[:, B + lo - 1:B + lo - 1 + w],
                                      s0=h_, s1=s_, imm2=2.0 * DX)
                nc.vector._custom_dve(BSQ,
                                      out=QD[:, lo:w + lo],
                                      in0=cur[:, B + lo:B + lo + w],
                                      in1=cur[:, B + lo + 1:B + lo + 1 + w],
                                      s0=h_, s1=s_)
                # interface precompute (reads OLD state only):
                # X = (ca/cb)*GR + b3; SELif scales by ifB = cb*m63 so
                # b4[63] gets ca*GR + cb*b3.
                nc.vector.scalar_tensor_tensor(
                    I1[:, lo:hi],
                    cur[:, 5 * B + lo:5 * B + hi], rab,
                    cur[:, 3 * B + lo:3 * B + hi],
                    OP.mult, OP.add)

                # S1 placed before CLIN/DLIN so the previous step's ghost
                # column refresh (PE+ACT) has extra slack before CLIN reads GL
                nc.vector.tensor_tensor(S1[:, lo:w + lo], QU[:, lo:w + lo],
                                        QD[:, lo:w + lo], OP.add)
                # CLIN: W stream = [GL|b1|b2|b3] contiguous
                nc.vector._custom_dve(CLIN,
                                      out=QL[:, lo:w + lo],
                                      in0=cur[:, B + lo:B + lo + w],
                                      in1=cur[:, lo:lo + w],
                                      s0=h_, s1=s_, imm2=2.0 * DX)
                # DLIN: E stream = [b2|b3|b4|GR] contiguous; carries phi
                # (includes raw Tc -> f32 output)
                nc.vector._custom_dve(DLIN,
                                      out=PR[:, lo:w + lo],
                                      in0=cur[:, B + lo:B + lo + w],
                                      in1=cur[:, 2 * B + lo:2 * B + lo + w],
                                      s0=h_, s1=s_, imm2=-4.0)
                nc.vector.tensor_tensor(S2[:, lo:w + lo], S1[:, lo:w + lo],
                                        QL[:, lo:w + lo], OP.add)
                nc.vector.tensor_tensor(nxt[:, B + lo:B + lo + w],
                                        S2[:, lo:w + lo], PR[:, lo:w + lo],
                                        OP.add)

                # boundary fixups, uniform order for every step: interface
                # blend and col-511 masked copy on DVE, col-0 copy on ACT
                # (hidden behind the DVE fixup tail), row copies on DVE.
                b4v = nxt[:, 4 * B + lo:4 * B + hi]
                nc.scalar.copy(nxt[0:1, B + lo:B + hi],
                               nxt[0:1, 2 * B + lo:2 * B + hi])
                nc.vector._custom_dve(SEL, out=b4v, in0=b4v,
                                      in1=I1[:, lo:hi], s0=ifC, s1=ifB)
                nc.vector.copy_predicated(b4v, mskt[:, lo:hi],
                                          nxt[:, 3 * B + lo:3 * B + hi])
                nx4 = nxt[:, B:5 * B].rearrange("p (b i) -> p b i", b=4)
                nc.vector.tensor_scalar(nx4[:, :, K:K + 1],
                                        nx4[:, :, bass.ds(sv_rtop, 1)],
                                        1.0, None, OP.mult)
                nc.vector.tensor_scalar(nx4[:, :, K + 63:K + 64],
                                        nx4[:, :, bass.ds(sv_rbot, 1)],
                                        1.0, None, OP.mult)

                # ghost row sync (blocking; gpsimd queue keeps Sync free).
                # Bands carry only the 4 state blocks; GL/GR are rebuilt by
                # the ghost-column matmuls placed AFTER the merge below.
                if t in nsync:
                    cc_out = cc_outs[t]
                    nc.sync.dma_start(cc_in[0], nx4[:, :, K:2 * K])
                    nc.sync.dma_start(cc_in[1], nx4[:, :, 64:64 + K])
                    nc.gpsimd.collective_compute(
                        "AllGather", OP.bypass,
                        replica_groups=[list(range(NCORES))],
                        ins=[cc_in[:]], outs=[cc_out[:]])
                    nc.gpsimd.dma_start(nx4[:, :, 0:K],
                                        cc_out[bass.ds(sv_prev, 128)])
                    nc.sync.dma_start(nx4[:, :, 64 + K:64 + 2 * K],
                                      cc_out[bass.ds(sv_next, 128)])

                # ghost column refresh via partition-shift matmuls, windowed
                # to the rows step t+1 will read.  At sync steps this reads
                # the merged tile, so the new GL/GR include fresh ghost rows.
                if t < NSTEPS:
                    m2 = (t % K) + 1
                    l2, h2 = m2, RS - m2
                    psR = pp.tile([128, B], f32, tag="psR")
                    psL = pp.tile([128, B], f32, tag="psL")
                    nc.tensor.matmul(psL[:, l2:h2], wl[:],
                                     nxt[:, 4 * B + l2:4 * B + h2],
                                     start=True, stop=True)
                    nc.tensor.matmul(psR[:, l2:h2], wr[:],
                                     nxt[:, B + l2:B + h2],
                                     start=True, stop=True)
                    nc.scalar.copy(nxt[:, l2:h2], psL[:, l2:h2])
                    nc.scalar.copy(nxt[:, 5 * B + l2:5 * B + h2],
                                   psR[:, l2:h2])

                # output: owned rows (the read has three steps of slack)
                nc.sync.dma_start(out_d[t - 1], nx4[:, :, K:K + 64])
    return nc


def _ensure_ntff_hook():
    """Provide antenv.axon_hooks (missing in this image) so bass_utils can
    NTFF-profile under axon."""
    import sys
    import types
    try:
        from antenv.axon_hooks import get_axon_ntff_profile_hook  # noqa: F401
        return
    except ImportError:
        pass
    mod = types.ModuleType("antenv.axon_hooks")
    mod._hook = None

    def set_axon_ntff_profile_hook(h):
        mod._hook = h

    def get_axon_ntff_profile_hook():
        return mod._hook

    mod.set_axon_ntff_profile_hook = set_axon_ntff_profile_hook
    mod.get_axon_ntff_profile_hook = get_axon_ntff_profile_hook
    sys.modules["antenv.axon_hooks"] = mod
    import antenv
    antenv.axon_hooks = mod
    try:
        from trn_agent_boot.trn_boot import _ntff_profile_via_ctypes
        hook = _ntff_profile_via_ctypes("/opt/axon/libaxon_pjrt.so")
        if hook is not None:
            mod._hook = hook
    except Exception:
        pass


def kernel(u0, k1, k2, alpha1, alpha2):
    global LAST_EXEC_NS, LAST_RESULT
    import concourse.bacc as bacc
    import concourse.bass as bass
    import concourse.tile as tile
    import concourse.mybir as mybir
    from concourse.bass_utils import run_bass_kernel_spmd

    u0 = np.asarray(u0, dtype=np.float32)
    k1f = float(np.asarray(k1).reshape(-1)[0])
    k2f = float(np.asarray(k2).reshape(-1)[0])
    a1f = float(np.asarray(alpha1).reshape(-1)[0])
    a2f = float(np.asarray(alpha2).reshape(-1)[0])

    dx2 = DX * DX
    ca, cb = k1f / (k1f + k2f), k2f / (k1f + k2f)

    nc = bacc.Bacc(
        "TRN2", target_bir_lowering=False, debug=False,
        num_devices=NCORES,
    )
    _build(nc, tile, mybir, bass)
    nc.compile()

    left = np.arange(128) < 64
    s = np.where(left, DT * a1f / dx2, DT * a2f / dx2).astype(np.float32)
    h = np.where(left, DT * k1f / (2 * DX), DT * k2f / (2 * DX)).astype(np.float32)
    WR = np.eye(128, k=-1, dtype=np.float32)   # out[m] = in[m+1]
    WL = np.eye(128, k=+1, dtype=np.float32)   # out[m] = in[m-1]

    m63 = (np.arange(128) == 63).astype(np.float32)
    m127 = (np.arange(128) == 127).astype(np.float32)
    in_maps = []
    for c in range(NCORES):
        ppc = np.zeros((128, 16), np.float32)
        ppc[:, 3] = s
        ppc[:, 4] = h
        ppc[:, 10] = m63 * cb              # ifB
        ppc[:, 11] = 1.0 - m63             # ifC
        ppc[:, 12] = m127                  # cD (copy_predicated mask source)
        ppc[:, 15] = ca / cb               # rab (interface ratio)
        prev_off = (2 * (c - 1) + 1) * 128 if c > 0 else 0
        next_off = (2 * (c + 1)) * 128 if c < NCORES - 1 else 0
        rtop = K + 1 if c == 0 else K
        rbot = K + 62 if c == NCORES - 1 else K + 63
        in_maps.append({
            "u0s": _pack_core(u0, c),
            "ppc": ppc,
            "wr": WR,
            "wl": WL,
            "nbrs": np.array([[prev_off, next_off]], dtype=np.uint32),
            "rsel": np.array([[rtop, rbot]], dtype=np.uint32),
            "msk": np.broadcast_to(m127[:, None].astype(np.uint8),
                                   (128, B)).copy(),
        })

    trace = os.environ.get("ADR_TRACE", "0") == "1"
    if trace:
        _ensure_ntff_hook()
    res = run_bass_kernel_spmd(
        nc, in_maps, core_ids=list(range(NCORES)), trace=trace)
    LAST_EXEC_NS = res.exec_time_ns
    LAST_RESULT = res

    full = np.zeros((NSTEPS, N, N), np.float32)
    for c in range(NCORES):
        arr = np.asarray(res.results[c]["out"]).reshape(NSTEPS, 128, 4, 64)
        full[:, 64 * c:64 * (c + 1), :] = (
            arr.transpose(0, 3, 1, 2).reshape(NSTEPS, 64, 512))
    return full
